# revision 6
# baseline (speedup 1.0000x reference)
"""DIST loss (hard CE + inter/intra Pearson distillation) on 8 Trainium2 cores.

Strategy: data-parallel over the batch dim (4096 rows -> 512 rows/core).
Each core streams its [512, 32000] f32 shard of z_s/z_t once from HBM,
computes exp(z)/2 on the ScalarE (caching fp8e4 exponentials in SBUF;
the /2 bias fold keeps values under the TRN fp8e4 +-240 max and cancels in
every softmax ratio; only log Zs needs a +ln2 host fix), then
produces:
  - per-row stats  [512, 5]: Zs, Zt, U11=sum(es^2), U22=sum(et^2), U12=sum(es*et)
    (U11 comes free from the ScalarE Square activation's accumulator;
     U22/U12 via VectorE halve-add + reduce)
  - per-column weighted partial sums (one slab per 128-row block):
    S1=sum(es/Zs), S2=sum(et/Zt), S11=sum(es^2/Zs^2), S22=sum(et^2/Zt^2),
    S12=sum(es*et/(Zs*Zt)) -- TensorE matmuls with zero-padded per-stat
    weight columns as the stationary operand; the three 512-col sub-matmuls
    of a chunk land at PSUM base partitions 0/32/64 of a single bank so one
    [69,512] VectorE copy evacuates the whole chunk.
The host sums the partial column stats over blocks/cores and finishes the
O(B + C) scalar math (Pearson means, label gather, log) in float64.
"""
import sys
import types
import numpy as np

sys.path.insert(0, "/opt/trn_rl_repo")

B, C = 4096, 32000
N_CORES = 8
R = B // N_CORES          # 512 rows per core
P = 128                   # partitions
NBLK = R // P             # 4 row blocks per core
CHUNK = 1536
CHUNKS = [(i * CHUNK, CHUNK) for i in range(20)] + [(20 * CHUNK, C - 20 * CHUNK)]
NCH = len(CHUNKS)
EPS = 1e-8

_built = None


def _install_ntff_shim():
    # antenv.axon_hooks is absent in this image; register the ctypes NTFF
    # hook so run_bass_kernel_spmd(trace=True) can profile under axon.
    try:
        import antenv
        import trn_agent_boot.trn_boot as tb
        if "antenv.axon_hooks" in sys.modules:
            return
        hook = tb._ntff_profile_via_ctypes("/opt/axon/libaxon_pjrt.so")
        mod = types.ModuleType("antenv.axon_hooks")
        mod.get_axon_ntff_profile_hook = lambda: hook
        mod.set_axon_ntff_profile_hook = lambda h: None
        antenv.axon_hooks = mod
        sys.modules["antenv.axon_hooks"] = mod
    except Exception:
        pass


def _sub_slices(cw):
    subs = []
    o = 0
    while o < cw:
        n = min(512, cw - o)
        subs.append((o, n))
        o += n
    return subs


def _build():
    from contextlib import ExitStack
    import concourse.bacc as bacc
    import concourse.tile as tile
    from concourse import mybir

    f32 = mybir.dt.float32
    bf16 = mybir.dt.bfloat16
    fp8 = mybir.dt.float8e4
    Exp = mybir.ActivationFunctionType.Exp
    Square = mybir.ActivationFunctionType.Square
    ADD = mybir.AluOpType.add
    AXF = mybir.AxisListType.X

    nc = bacc.Bacc("TRN2", target_bir_lowering=False, debug=False)
    zs_d = nc.dram_tensor("z_s", [R, C], f32, kind="ExternalInput")
    zt_d = nc.dram_tensor("z_t", [R, C], f32, kind="ExternalInput")
    # [block, chunk, psum partition, 512]: rows 32s..32s+4 hold stats 0..4 of
    # sub-matmul s; everything else is don't-care filler the host skips.
    col_d = nc.dram_tensor("colstats", [NBLK, NCH, 69, 512], f32,
                           kind="ExternalOutput")
    row_d = nc.dram_tensor("rowstats", [R, 8], f32, kind="ExternalOutput")

    GRP = 2  # chunks per PE burst group (product tiles buffered GRP+1 deep)

    with tile.TileContext(nc) as tc, ExitStack() as ctx:
        zin = ctx.enter_context(tc.tile_pool(name="zin", bufs=3))
        esp = ctx.enter_context(tc.tile_pool(name="esp", bufs=NCH + 8))
        etp = ctx.enter_context(tc.tile_pool(name="etp", bufs=NCH + 8))
        prod = ctx.enter_context(tc.tile_pool(name="prod", bufs=3 * (GRP + 1)))
        halfp = ctx.enter_context(tc.tile_pool(name="halfp", bufs=4))
        statp = ctx.enter_context(tc.tile_pool(name="stat", bufs=4))
        small = ctx.enter_context(tc.tile_pool(name="small", bufs=2))
        psump = ctx.enter_context(tc.tile_pool(name="psum", bufs=6, space="PSUM"))

        lnh = small.tile([P, 1], f32, tag="lnh")
        nc.vector.memset(lnh[:], float(np.log(0.5)))
        for b in range(NBLK):
            r0 = b * P
            zsp = small.tile([P, NCH], f32, tag="zsp")
            ztp = small.tile([P, NCH], f32, tag="ztp")
            u11p = small.tile([P, NCH], f32, tag="u11p")
            u22p = small.tile([P, NCH], f32, tag="u22p")
            u12p = small.tile([P, NCH], f32, tag="u12p")

            es_tiles = []
            et_tiles = []
            prod_tiles = {}
            for ci, (c0, cw) in enumerate(CHUNKS):
                zs = zin.tile([P, cw], f32, tag="zin")
                nc.sync.dma_start(zs[:], zs_d[r0:r0 + P, c0:c0 + cw])
                es = esp.tile([P, cw], fp8, tag="es")
                nc.scalar.activation(es[:], zs[:], Exp, bias=lnh[:, 0:1],
                                     accum_out=zsp[:, ci:ci + 1])
                zt = zin.tile([P, cw], f32, tag="zin")
                nc.sync.dma_start(zt[:], zt_d[r0:r0 + P, c0:c0 + cw])
                et = etp.tile([P, cw], fp8, tag="et")
                nc.scalar.activation(et[:], zt[:], Exp, bias=lnh[:, 0:1],
                                     accum_out=ztp[:, ci:ci + 1])
                es_tiles.append(es)
                et_tiles.append(et)

            rs = small.tile([P, 8], f32, tag="rs")
            nc.vector.tensor_reduce(rs[:, 0:1], zsp[:, 0:NCH], axis=AXF, op=ADD)
            nc.vector.tensor_reduce(rs[:, 1:2], ztp[:, 0:NCH], axis=AXF, op=ADD)
            w1 = small.tile([P, 1], f32, tag="w1")
            nc.vector.reciprocal(w1[:], rs[:, 0:1])
            w2 = small.tile([P, 1], f32, tag="w2")
            nc.vector.reciprocal(w2[:], rs[:, 1:2])
            # Stat k's weights live in column k of an otherwise-zero [P, 5]
            # stationary tile, so 5 accumulating matmuls (one per stat, each
            # with its own rhs) build a [5, n] PSUM block at base partition
            # 0/32/64 (one per sub-matmul of the chunk).
            W_tiles = []
            for k in range(5):
                Wk = small.tile([P, 5], bf16, tag=f"W{k}")
                nc.vector.memset(Wk[:], 0.0)
                W_tiles.append(Wk)
            nc.vector.tensor_copy(W_tiles[0][:, 0:1], w1[:])
            nc.vector.tensor_copy(W_tiles[1][:, 1:2], w2[:])
            nc.vector.tensor_mul(W_tiles[2][:, 2:3], w1[:], w1[:])
            nc.vector.tensor_mul(W_tiles[3][:, 3:4], w2[:], w2[:])
            nc.vector.tensor_mul(W_tiles[4][:, 4:5], w1[:], w2[:])

            def emit_products(ci):
                c0, cw = CHUNKS[ci]
                es, et = es_tiles[ci], et_tiles[ci]
                p11 = prod.tile([P, cw], bf16, tag="prod")
                nc.scalar.activation(p11[:], es[:], Square,
                                     accum_out=u11p[:, ci:ci + 1])
                p22 = prod.tile([P, cw], bf16, tag="prod")
                nc.vector.tensor_mul(p22[:], et[:], et[:])
                p12 = prod.tile([P, cw], bf16, tag="prod")
                nc.vector.tensor_mul(p12[:], es[:], et[:])
                h = cw // 2
                h22 = halfp.tile([P, h], bf16, tag="half")
                nc.vector.tensor_add(h22[:], p22[:, 0:h], p22[:, h:cw])
                nc.vector.tensor_reduce(u22p[:, ci:ci + 1], h22[:], axis=AXF, op=ADD)
                h12 = halfp.tile([P, h], bf16, tag="half")
                nc.vector.tensor_add(h12[:], p12[:, 0:h], p12[:, h:cw])
                nc.vector.tensor_reduce(u12p[:, ci:ci + 1], h12[:], axis=AXF, op=ADD)
                prod_tiles[ci] = (p11, p22, p12)

            def emit_matmuls(ci):
                c0, cw = CHUNKS[ci]
                es, et = es_tiles[ci], et_tiles[ci]
                p11, p22, p12 = prod_tiles.pop(ci)
                rhs_list = [es, et, p11, p22, p12]
                ps = psump.tile([69, 512], f32, tag="ps")
                for s, (o, n) in enumerate(_sub_slices(cw)):
                    for k in range(5):
                        nc.tensor.matmul(ps[32 * s:32 * s + 5, 0:n],
                                         W_tiles[k][:, 0:5],
                                         rhs_list[k][:, o:o + n],
                                         start=(k == 0), stop=(k == 4))
                st = statp.tile([69, 512], f32, tag="st")
                if ci % 2 == 0:
                    nc.vector.tensor_copy(st[:], ps[:])
                else:
                    nc.scalar.copy(st[:], ps[:])
                nc.sync.dma_start(col_d[b, ci], st[:])

            for g0 in range(0, NCH, GRP):
                group = range(g0, min(g0 + GRP, NCH))
                for ci in group:
                    emit_products(ci)
                for ci in group:
                    emit_matmuls(ci)

            nc.vector.tensor_reduce(rs[:, 2:3], u11p[:, 0:NCH], axis=AXF, op=ADD)
            nc.vector.tensor_reduce(rs[:, 3:4], u22p[:, 0:NCH], axis=AXF, op=ADD)
            nc.vector.tensor_reduce(rs[:, 4:5], u12p[:, 0:NCH], axis=AXF, op=ADD)
            nc.sync.dma_start(row_d[r0:r0 + P, 0:5], rs[:, 0:5])

    nc.compile()
    return nc


def _get_built():
    global _built
    if _built is None:
        _install_ntff_shim()
        _built = _build()
    return _built


def _unpack_col(colstats):
    """colstats [NBLK, NCH, 69, 512] (f32, already summed over cores ok) ->
    [5, C] float64 column stats."""
    acc = colstats.astype(np.float64).sum(axis=0)   # [NCH, 69, 512]
    col = np.zeros((5, C), np.float64)
    for ci, (c0, cw) in enumerate(CHUNKS):
        for s, (o, n) in enumerate(_sub_slices(cw)):
            col[:, c0 + o:c0 + o + n] += acc[ci, 32 * s:32 * s + 5, 0:n]
    return col


def run_sharded(z_s, z_t, trace=False, tmpdir=None):
    """Run the device program; returns (colstats_sum [5, C] f64,
    rowstats [B, 5] f64, BassKernelResults)."""
    from concourse.bass_utils import run_bass_kernel_spmd

    nc = _get_built()
    z_s = np.ascontiguousarray(np.asarray(z_s, dtype=np.float32))
    z_t = np.ascontiguousarray(np.asarray(z_t, dtype=np.float32))
    in_maps = [
        {"z_s": z_s[i * R:(i + 1) * R], "z_t": z_t[i * R:(i + 1) * R]}
        for i in range(N_CORES)
    ]
    res = run_bass_kernel_spmd(nc, in_maps, core_ids=list(range(N_CORES)),
                               trace=trace, tmpdir=tmpdir)
    col = np.zeros((5, C), np.float64)
    rows = []
    for i in range(N_CORES):
        col += _unpack_col(res.results[i]["colstats"])
        rows.append(res.results[i]["rowstats"][:, :5].astype(np.float64))
    return col, np.concatenate(rows, axis=0), res


def kernel(z_s, z_t, labels):
    col, rowstats, _ = run_sharded(z_s, z_t)
    return _finish(np.asarray(z_s), np.asarray(labels), col, rowstats)


def _finish(z_s, labels, col, rowstats):
    Zs, Zt, U11, U22, U12 = rowstats.T
    invC = 1.0 / C
    # inter: Pearson over classes per row (softmax rows have mean 1/C)
    num = U12 / (Zs * Zt) - invC
    vs = U11 / (Zs * Zs) - invC
    vt = U22 / (Zt * Zt) - invC
    corr = num / (np.sqrt(vs) * np.sqrt(vt) + EPS)
    inter = 1.0 - corr.mean()
    # intra: Pearson over samples per column
    S1, S2, S11, S22, S12 = col
    numc = S12 - S1 * S2 / B
    vsc = S11 - S1 * S1 / B
    vtc = S22 - S2 * S2 / B
    corrc = numc / (np.sqrt(vsc) * np.sqrt(vtc) + EPS)
    intra = 1.0 - corrc.mean()
    # hard CE: mean(logsumexp(z_s) - z_s[label])
    lab = np.asarray(labels).astype(np.int64).ravel()
    zl = z_s[np.arange(B), lab].astype(np.float64)
    hard = (np.log(2.0 * Zs) - zl).mean()
    return np.float32(hard + inter + intra)
